# revision 20
# baseline (speedup 1.0000x reference)
"""Multi-Head Latent Attention (MLA) Bass kernel for 8 Trainium2 NeuronCores.

Sharding:
  - latent projections (d_kv, d_q): sequence-sharded (BS/8 rows per core),
    AllGathers (Shared-output HBM fast path) replicate the latents.
  - up-projections (u_k, u_v, u_q, qr) + attention: head-sharded, 2 heads/core.
  - context: per-head AllToAlls (head-0 A2A overlapped with head-1 attention,
    head-1 A2A overlapped with the first half of out_proj) re-shard to
    sequence; out_proj sequence-parallel.
Precision: bf16 matmuls everywhere except the QK^T scores, which run as fp8e4
DoubleRow matmuls (K=192 contraction in ONE instruction, 2x bf16 FLOPs).
K/Q operands are stored fp8 with power-of-2 scale-up (SK/SQ) to dodge e4m3's
subnormal range; the descale folds into the softmax exp's scale. The V path
and out_proj stay >=bf16 since their error hits the output directly.
Softmax denominators ride as a ones-column in the V tiles; rope is two
projection chains combined with (pre-scaled) cos/sin tables on the DVE.
"""
import sys
import os

for _p in ("/opt/trn_rl_repo", "/root/.axon_site/_ro/trn_rl_repo"):
    if os.path.isdir(_p) and _p not in sys.path:
        sys.path.insert(0, _p)

import math
import numpy as np
import ml_dtypes

BF = ml_dtypes.bfloat16
E4M3 = ml_dtypes.float8_e4m3

import concourse.bacc as bacc
import concourse.mybir as mybir
from concourse import tile
from concourse.bass_utils import run_bass_kernel_spmd
from concourse.masks import make_identity

# problem dims (hardcoded)
B, S, H, Dh, Dr, HID, C = 2, 2048, 16, 128, 64, 2048, 512
BS = B * S                      # 4096
NCORES = 8
H_LOC = H // NCORES             # 2
S_LOC = BS // NCORES            # 512
SCALE = 1.0 / math.sqrt(Dh + Dr)
SK = 8.0                        # fp8 store scale, k side
SQ = 64.0                       # fp8 store scale, q side (on top of SCALE)

F32 = mybir.dt.float32
F16 = mybir.dt.float16
BF16 = mybir.dt.bfloat16
F8 = mybir.dt.float8e4
DR = mybir.MatmulPerfMode.DoubleRow
ADD = mybir.AluOpType.add
MULT = mybir.AluOpType.mult
EXP = mybir.ActivationFunctionType.Exp
IDENT = mybir.ActivationFunctionType.Identity
GROUPS = [list(range(NCORES))]

_CACHE = {}


def _phase_a(tc, nc, io, kv_ag_i, kv_ag_o, ql_ag_i, ql_ag_o):
    """Sequence-sharded latent projections + AllGathers (both bf16)."""
    with tc.tile_pool(name="phA", bufs=1) as phA, \
         tc.tile_pool(name="phAb", bufs=4) as phAb, \
         tc.tile_pool(name="psA", bufs=1, space="PSUM") as psA:
        xts, wkv, wql = [], [], []
        for pr in range(8):
            xt = phA.tile([128, 2, S_LOC], BF16, tag=f"xt{pr}",
                          name=f"xt{pr}")
            nc.sync.dma_start(out=xt[:], in_=io["xT_in"][pr])
            xts.extend([xt[:, 0, :], xt[:, 1, :]])
            w = phA.tile([128, 2, C], BF16, tag=f"wk{pr}", name=f"wk{pr}")
            nc.sync.dma_start(out=w[:], in_=io["wdkvT"][pr])
            wkv.extend([w[:, 0, :], w[:, 1, :]])
        for pr in range(8):
            w = phA.tile([128, 2, C], BF16, tag=f"wq{pr}", name=f"wq{pr}")
            nc.sync.dma_start(out=w[:], in_=io["wdqT"][pr])
            wql.extend([w[:, 0, :], w[:, 1, :]])

        # ht-outer accumulation: matmuls start as soon as tiles land
        for wfull, bdram, agi, ago in ((wkv, "bdkv", kv_ag_i, kv_ag_o),
                                       (wql, "bdq", ql_ag_i, ql_ag_o)):
            pss = [psA.tile([128, S_LOC], F32, tag=f"ps{bdram}{ct}",
                            name=f"ps{ct}") for ct in range(4)]
            for ht in range(16):
                for ct in range(4):
                    nc.tensor.matmul(pss[ct][:],
                                     wfull[ht][:, ct * 128:(ct + 1) * 128],
                                     xts[ht], start=(ht == 0),
                                     stop=(ht == 15))
            for ct in range(4):
                bt = phAb.tile([128, 1], F32, tag="blat", name="blat")
                nc.sync.dma_start(out=bt[:], in_=io[bdram][ct])
                lat = phAb.tile([128, S_LOC], BF16, tag="lat", name="lat")
                nc.vector.tensor_scalar_add(lat[:], pss[ct][:], bt[:])
                nc.scalar.dma_start(out=agi[ct], in_=lat[:])
            nc.gpsimd.collective_compute(
                "AllGather", mybir.AluOpType.bypass, replica_groups=GROUPS,
                ins=[agi.opt()], outs=[ago.opt()])


def _phase_b(tc, nc, io, ones_row, kT8, qT8, v_sb, kv_ag_o, ql_ag_o):
    """Head-sharded bf16 up-projections + RoPE; K/Q stored fp8 (scaled)."""
    with tc.tile_pool(name="phB", bufs=1) as phB, \
         tc.tile_pool(name="phBt", bufs=3) as phBt, \
         tc.tile_pool(name="psB", bufs=2, space="PSUM") as psB:
        tabs = {}
        for nm in ("c1k", "c2k", "c1q", "c2q"):
            t = phB.tile([128, S], F16, tag=nm, name=nm)
            nc.scalar.dma_start(out=t[:], in_=io[nm][:])
            tabs[nm] = t
        buv_s = phB.tile([1, 256], BF16, tag="buv", name="buv_s")
        nc.scalar.dma_start(out=buv_s[:], in_=io["buv"][:])
        bias_r = {}
        for nm in ("bAq", "bBq", "bAk", "bBk"):
            bs_ = phB.tile([128, 1], F32, tag=nm, name=nm + "s")
            nc.scalar.dma_start(out=bs_[:], in_=io[nm][:])
            bias_r[nm] = bs_
        upw = {}
        for nm, w_ in (("uk", 256), ("uq", 256), ("wrAq", 128),
                       ("wrBq", 128), ("wrAk", 128), ("wrBk", 128)):
            wt = phB.tile([128, 4, w_], BF16, tag=nm, name=nm)
            nc.scalar.dma_start(out=wt[:], in_=io[nm + "T"][:])
            upw[nm] = [wt[:, ct, :] for ct in range(4)]
        uvt = phB.tile([128, 4, 256], BF16, tag="uv", name="uvt")
        nc.scalar.dma_start(out=uvt[:], in_=io["uvT"][:])
        uvs = [uvt[:, ct, :] for ct in range(4)]
        bukq = {}
        for nm in ("buk", "buq"):
            tl = []
            for h in range(2):
                bt_ = phB.tile([128, 1], F32, tag=f"{nm}{h}", name=f"{nm}{h}")
                nc.scalar.dma_start(out=bt_[:], in_=io[nm][h])
                tl.append(bt_)
            bukq[nm] = tl
        # V bias broadcast tile via rank-1 matmul
        psvb = psB.tile([128, 256], F32, tag="psV", name="psvb")
        nc.tensor.matmul(psvb[:], ones_row[:], buv_s[:], start=True, stop=True)
        vbias = phB.tile([128, 256], F16, tag="vbias", name="vbias")
        nc.vector.tensor_copy(vbias[:], psvb[:])

        def rproj(ps_tag, wa, src):
            ps_ = psB.tile([128, 512], F32, tag=ps_tag, name=ps_tag)
            for ct in range(4):
                nc.tensor.matmul(ps_[:], upw[wa][ct], src[ct][:],
                                 start=(ct == 0), stop=(ct == 3))
            return ps_

        def rope_write(dsts, wa, wb, ba, bb, c1, c2, src, pos):
            psa_ = rproj("psRA", wa, src)
            psb_ = rproj("psRB", wb, src)
            t1 = phBt.tile([128, 512], F32, tag="t1", bufs=2, name="t1")
            nc.vector.scalar_tensor_tensor(
                t1[:], psa_[:], bias_r[ba][:], tabs[c1][:, pos], ADD, MULT)
            t2 = phBt.tile([128, 512], F32, tag="t2", bufs=2, name="t2")
            nc.vector.scalar_tensor_tensor(
                t2[:], psb_[:], bias_r[bb][:], tabs[c2][:, pos], ADD, MULT)
            # head h's rope dims live in partition band [64h, 64h+64) of
            # plane 1 (the other band is zeroed), keeping DVE partition
            # bases aligned
            for h in range(2):
                band = slice(h * 64, (h + 1) * 64)
                nc.vector.tensor_add(dsts[h], t1[band, :], t2[band, :])

        # ---- pass 1: kv-dependent (k_c, rope-k, V) ----
        for j2 in range(8):
            sl = slice(j2 * 512, (j2 + 1) * 512)
            pos = slice((j2 % 4) * 512, (j2 % 4) * 512 + 512)
            kv_bf = []
            for ct in range(4):
                kt_ = phBt.tile([128, 512], BF16, tag=f"kv{ct}", bufs=2,
                                name=f"kv{ct}")
                nc.sync.dma_start(out=kt_[:], in_=kv_ag_o[j2, ct])
                kv_bf.append(kt_)
            for h in range(2):
                hc = slice(h * 128, (h + 1) * 128)
                ps = psB.tile([128, 512], F32, tag="psKC", name="pskc")
                for ct in range(4):
                    nc.tensor.matmul(ps[:], upw["uk"][ct][:, hc],
                                     kv_bf[ct][:], start=(ct == 0),
                                     stop=(ct == 3))
                nc.scalar.activation(kT8[h][:, 0, sl], ps[:], IDENT,
                                     bias=bukq["buk"][h][:], scale=SK)
            rope_write([kT8[h][h * 64:(h + 1) * 64, 1, sl] for h in range(2)],
                       "wrAk", "wrBk", "bAk", "bBk", "c1k", "c2k",
                       kv_bf, pos)
            for ss in range(4):
                psv = psB.tile([128, 256], F32, tag="psV", name="psv")
                ssl = slice(ss * 128, (ss + 1) * 128)
                for ct in range(4):
                    nc.tensor.matmul(psv[:], kv_bf[ct][:, ssl], uvs[ct],
                                     start=(ct == 0), stop=(ct == 3))
                st = j2 * 4 + ss
                nc.vector.tensor_add(v_sb[:, st, 0:128], psv[:, 0:128],
                                     vbias[:, 0:128])
                nc.vector.tensor_add(v_sb[:, st, 129:257], psv[:, 128:256],
                                     vbias[:, 128:256])

        # ---- pass 2: ql-dependent (q_c, rope-q) ----
        for j2 in range(8):
            sl = slice(j2 * 512, (j2 + 1) * 512)
            pos = slice((j2 % 4) * 512, (j2 % 4) * 512 + 512)
            ql_bf = []
            for ct in range(4):
                qt_ = phBt.tile([128, 512], BF16, tag=f"ql{ct}", bufs=2,
                                name=f"ql{ct}")
                nc.sync.dma_start(out=qt_[:], in_=ql_ag_o[j2, ct])
                ql_bf.append(qt_)
            for h in range(2):
                hc = slice(h * 128, (h + 1) * 128)
                ps = psB.tile([128, 512], F32, tag="psKC", name="psqc")
                for ct in range(4):
                    nc.tensor.matmul(ps[:], upw["uq"][ct][:, hc],
                                     ql_bf[ct][:], start=(ct == 0),
                                     stop=(ct == 3))
                nc.scalar.activation(qT8[h][:, 0, sl], ps[:], IDENT,
                                     bias=bukq["buq"][h][:], scale=SQ)
            rope_write([qT8[h][h * 64:(h + 1) * 64, 1, sl] for h in range(2)],
                       "wrAq", "wrBq", "bAq", "bBq", "c1q", "c2q",
                       ql_bf, pos)


def _phase_c(tc, nc, kT8, qT8, v_sb, ident, a2a_i, a2a_o, phD0, csl):
    """Per-head attention; per-head AllToAll overlaps the other head/out."""
    with tc.tile_pool(name="phC", bufs=1) as phC, \
         tc.tile_pool(name="psC", bufs=1, space="PSUM") as psC:

        def flush_ctx(pend):
            h, g, ctxns = pend
            for sqs in range(4):
                nc.sync.dma_start(
                    out=a2a_i[h][g, sqs * 128:(sqs + 1) * 128, :],
                    in_=ctxns[sqs][:])

        pending = None
        for h in range(2):
            for g in range(8):
                b, sqb = g // 4, g % 4
                qsl = slice(b * S + sqb * 512, b * S + sqb * 512 + 512)
                probs = phC.tile([128, 16, 512], F16, tag="probs", bufs=2,
                                 name="probs")
                for st2 in range(8):
                    ps2 = psC.tile([128, 2, 512], F32, tag="psS", bufs=3,
                                   name="ps2")
                    for p in range(2):
                        skt = st2 * 2 + p
                        ksl = slice(b * S + skt * 128, b * S + skt * 128 + 128)
                        nc.tensor.matmul(ps2[:, p, :], kT8[h][:, :, ksl],
                                         qT8[h][:, :, qsl],
                                         start=True, stop=True, perf_mode=DR)
                    nc.scalar.activation(probs[:, st2 * 2:st2 * 2 + 2, :],
                                         ps2[:], EXP, scale=1.0 / (SK * SQ))
                ctxns = []
                for sqs in range(4):
                    psx = psC.tile([128, 132], F32, tag="psX", bufs=2,
                                   name="psx")
                    for skt in range(16):
                        vt = b * 16 + skt
                        nc.tensor.matmul(
                            psx[:, 0:129],
                            probs[:, skt, sqs * 128:(sqs + 1) * 128],
                            v_sb[:, vt, h * 129:h * 129 + 129],
                            start=(skt == 0), stop=(skt == 15))
                    rec = phC.tile([128, 1], F32, tag="rec", bufs=4,
                                   name="rec")
                    nc.vector.reciprocal(rec[:], psx[:, 128:129])
                    ctxn = phC.tile([128, 128], F16, tag="ctxn", bufs=12,
                                    name="ctxn")
                    nc.vector.tensor_scalar_mul(ctxn[:], psx[:, 0:128],
                                                rec[:])
                    ctxns.append(ctxn)
                if pending is not None:
                    flush_ctx(pending)
                pending = (h, g, ctxns)
            flush_ctx(pending)
            pending = None
            nc.gpsimd.collective_compute(
                "AllToAll", mybir.AluOpType.bypass, replica_groups=GROUPS,
                ins=[a2a_i[h].opt()], outs=[a2a_o[h].opt()])
            for c in range(8):
                cf = phD0.tile([128, S_LOC], F16, tag=f"cf{2 * c + h}",
                               name=f"cf{2 * c + h}")
                nc.scalar.dma_start_transpose(out=cf[:], in_=a2a_o[h][c])
                csl[2 * c + h] = cf


def _phase_d(tc, nc, io, wo, csl):
    """Sequence-parallel out projection; first half of the contraction uses
    only head-0 context so it overlaps the head-1 AllToAll."""
    with tc.tile_pool(name="phDo", bufs=4) as phDo, \
         tc.tile_pool(name="psD", bufs=1, space="PSUM") as psD:
        psos = [psD.tile([128, 512], F32, tag=f"psO{i}", name=f"psO{i}")
                for i in range(8)]
        for i in range(8):
            ot, ssub = i // 4, i % 4
            ssl = slice(ssub * 128, (ssub + 1) * 128)
            osl = slice(ot * 512, (ot + 1) * 512)
            for dht in range(0, 16, 2):
                nc.tensor.matmul(psos[i][:], csl[dht][:, ssl],
                                 wo[dht][:, osl],
                                 start=(dht == 0), stop=False)
        for i in range(8):
            ot, ssub = i // 4, i % 4
            ssl = slice(ssub * 128, (ssub + 1) * 128)
            osl = slice(ot * 512, (ot + 1) * 512)
            for dht in range(1, 16, 2):
                nc.tensor.matmul(psos[i][:], csl[dht][:, ssl],
                                 wo[dht][:, osl],
                                 start=False, stop=(dht == 15))
            osb = phDo.tile([128, 512], F32, tag="osb", name="osb")
            nc.scalar.copy(osb[:], psos[i][:])
            nc.sync.dma_start(out=io["out_sl"][ssl, osl], in_=osb[:])
        for i in range(8):
            ot, ssub = 2 + i // 4, i % 4
            ssl = slice(ssub * 128, (ssub + 1) * 128)
            osl2 = slice((ot - 2) * 512, (ot - 1) * 512)
            osl = slice(ot * 512, (ot + 1) * 512)
            pso = psD.tile([128, 512], F32, tag=f"psO{i}", name="psoB")
            for dht in range(16):
                nc.tensor.matmul(pso[:], csl[dht][:, ssl],
                                 wo[dht][:, 1024 + (ot - 2) * 512:
                                         1024 + (ot - 1) * 512],
                                 start=(dht == 0), stop=(dht == 15))
            osb = phDo.tile([128, 512], F32, tag="osb", name="osb")
            nc.scalar.copy(osb[:], pso[:])
            nc.sync.dma_start(out=io["out_sl"][ssl, osl], in_=osb[:])


def _build_program():
    nc = bacc.Bacc("TRN2", target_bir_lowering=False, debug=False,
                   num_devices=NCORES)

    def din(name, shape, dtype):
        return nc.dram_tensor(name, shape, dtype, kind="ExternalInput")

    io = {
        "xT_in": din("xT_loc", [8, 128, 2, S_LOC], BF16),
        "wdkvT": din("wdkvT", [8, 128, 2, C], BF16),
        "wdqT": din("wdqT", [8, 128, 2, C], BF16),
        "bdkv": din("bdkv", [4, 128, 1], F32),
        "bdq": din("bdq", [4, 128, 1], F32),
        "ukT": din("ukT", [128, 4, 256], BF16),
        "uqT": din("uqT", [128, 4, 256], BF16),
        "uvT": din("uvT", [128, 4, 256], BF16),
        "buk": din("buk", [2, 128, 1], F32),
        "buq": din("buq", [2, 128, 1], F32),
        "buv": din("buv", [1, 256], BF16),
        "wrAqT": din("wrAqT", [128, 4, 128], BF16),
        "wrBqT": din("wrBqT", [128, 4, 128], BF16),
        "wrAkT": din("wrAkT", [128, 4, 128], BF16),
        "wrBkT": din("wrBkT", [128, 4, 128], BF16),
        "bAq": din("bAq", [128, 1], F32),
        "bBq": din("bBq", [128, 1], F32),
        "bAk": din("bAk", [128, 1], F32),
        "bBk": din("bBk", [128, 1], F32),
        "c1k": din("c1k", [128, S], F16),
        "c2k": din("c2k", [128, S], F16),
        "c1q": din("c1q", [128, S], F16),
        "c2q": din("c2q", [128, S], F16),
        "woT": din("woT", [16, 128, HID], BF16),
        "out_sl": nc.dram_tensor("out_slice", [S_LOC, HID], F32,
                                 kind="ExternalOutput"),
    }

    with tile.TileContext(nc) as tc:
        with tc.tile_pool(name="dram", bufs=1, space="DRAM") as dram:
            kv_ag_i = dram.tile([4, 128, S_LOC], BF16)
            kv_ag_o = dram.tile([NCORES, 4, 128, S_LOC], BF16,
                                addr_space="Shared", name="kvago")
            ql_ag_i = dram.tile([4, 128, S_LOC], BF16)
            ql_ag_o = dram.tile([NCORES, 4, 128, S_LOC], BF16,
                                addr_space="Shared", name="qlago")
            a2a_i = [dram.tile([NCORES, S_LOC, 128], F16, name=f"a2ai{h}")
                     for h in range(2)]
            a2a_o = [dram.tile([NCORES, S_LOC, 128], F16, name=f"a2ao{h}")
                     for h in range(2)]
            warm_i = dram.tile([1, 64], F32, name="warm_i")
            warm_o = dram.tile([NCORES, 1, 64], F32, name="warm_o")

            with tc.tile_pool(name="const", bufs=1) as const:
                ones_row = const.tile([1, 128], BF16)
                nc.vector.memset(ones_row[:], 1.0)
                ident = const.tile([128, 128], F16)
                make_identity(nc, ident[:])
                # tiny warmup AllGather: pays the one-time CC setup (~28us)
                # while phase A computes
                warm_sb = const.tile([1, 64], F32)
                nc.vector.memset(warm_sb[:], 0.0)
                nc.sync.dma_start(out=warm_i[:], in_=warm_sb[:])
                nc.gpsimd.collective_compute(
                    "AllGather", mybir.AluOpType.bypass,
                    replica_groups=GROUPS,
                    ins=[warm_i.opt()], outs=[warm_o.opt()])

                _phase_a(tc, nc, io, kv_ag_i, kv_ag_o, ql_ag_i, ql_ag_o)

                with tc.tile_pool(name="attn", bufs=1) as attn:
                    # [feat 128, plane 2, seq BS]; plane0 = k_c/q_c dims,
                    # plane1 band [64h, 64h+64) = head h's rope dims
                    kT8 = [attn.tile([128, 2, BS], F8, tag=f"kT8{h}",
                                     name=f"kT8{h}") for h in range(2)]
                    qT8 = [attn.tile([128, 2, BS], F8, tag=f"qT8{h}",
                                     name=f"qT8{h}") for h in range(2)]
                    v_sb = attn.tile([128, 32, 258], F16, tag="v",
                                     name="v_sb")
                    # zero unused rope bands (uninitialized fp8 bytes can
                    # encode NaN; NaN*0 poisons the PE accumulation) and set
                    # the denominator ones-columns
                    nc.vector.memset(qT8[0][64:128, 1, :], 0.0)
                    nc.vector.memset(qT8[1][0:64, 1, :], 0.0)
                    nc.vector.memset(kT8[0][64:128, 1, :], 0.0)
                    nc.vector.memset(kT8[1][0:64, 1, :], 0.0)
                    nc.vector.memset(v_sb[:, :, 128:129], 1.0)
                    nc.vector.memset(v_sb[:, :, 257:258], 1.0)

                    _phase_b(tc, nc, io, ones_row, kT8, qT8, v_sb,
                             kv_ag_o, ql_ag_o)

                    with tc.tile_pool(name="phD0", bufs=1) as phD0:
                        wo_full = []
                        for dht in range(16):
                            wo = phD0.tile([128, 2048], BF16, tag=f"wo{dht}",
                                           name=f"wo{dht}")
                            nc.sync.dma_start(out=wo[:], in_=io["woT"][dht])
                            wo_full.append(wo)
                        csl = [None] * 16
                        _phase_c(tc, nc, kT8, qT8, v_sb, ident, a2a_i, a2a_o, phD0, csl)
                        _phase_d(tc, nc, io, wo_full, csl)

    nc.compile()
    return nc


def _host_prep(inputs):
    """Build per-core input maps from the full problem inputs."""
    x = np.asarray(inputs["x"], np.float32)
    xT = np.ascontiguousarray(x.reshape(BS, HID).T)            # [HID, BS]

    def pack_hid(wT):
        # [HID, N] -> [8 pair, 128 part, 2 plane, N] (2KB+ DMA lines)
        n = wT.shape[1]
        return np.ascontiguousarray(
            wT.reshape(8, 2, 128, n).transpose(0, 2, 1, 3))

    def pack_c(wT):
        # [C, N] -> [128 part, 4 ct, N]
        n = wT.shape[1]
        return np.ascontiguousarray(wT.reshape(4, 128, n).transpose(1, 0, 2))

    wdkvT = pack_hid(np.asarray(inputs["d_kv_w"], np.float32).T).astype(BF)
    wdqT = pack_hid(np.asarray(inputs["d_q_w"], np.float32).T).astype(BF)
    bdkv = np.asarray(inputs["d_kv_b"], np.float32).reshape(4, 128, 1)
    bdq = np.asarray(inputs["d_q_b"], np.float32).reshape(4, 128, 1)

    uk3 = np.asarray(inputs["u_k_w"], np.float32).reshape(H, Dh, C)
    uq3 = np.asarray(inputs["u_q_w"], np.float32).reshape(H, Dh, C) * SCALE
    uv3 = np.asarray(inputs["u_v_w"], np.float32).reshape(H, Dh, C)
    buk2 = np.asarray(inputs["u_k_b"], np.float32).reshape(H, Dh) * SK
    buq2 = (np.asarray(inputs["u_q_b"], np.float32).reshape(H, Dh)
            * SCALE * SQ)
    buv2 = np.asarray(inputs["u_v_b"], np.float32).reshape(H, Dh)
    qr3 = np.asarray(inputs["qr_w"], np.float32).reshape(H, Dr, C)
    bqr2 = np.asarray(inputs["qr_b"], np.float32).reshape(H, Dr)

    # rope tables (positions 0..S-1), pre-scaled by the fp8 store scales
    i32 = np.arange(32, dtype=np.float32)
    inv_freq = (10000.0 ** (-(2.0 * i32) / Dr)).astype(np.float32)  # [32]
    pos = np.arange(S, dtype=np.float32)
    ang = pos[None, :] * inv_freq[:, None]                     # [32, S]
    cos, sin = np.cos(ang), np.sin(ang)
    c1 = np.concatenate([cos, sin, cos, sin], 0).astype(np.float32)
    c2 = np.concatenate([-sin, cos, -sin, cos], 0).astype(np.float32)

    woT = np.ascontiguousarray(
        np.asarray(inputs["out_w"], np.float32).T.astype(BF).reshape(
            16, 128, HID))

    in_maps = []
    for j in range(NCORES):
        hs = [2 * j, 2 * j + 1]
        xs = xT[:, j * S_LOC:(j + 1) * S_LOC]
        ukT_l = uk3[hs].transpose(2, 0, 1).reshape(C, 256)
        uqT_l = uq3[hs].transpose(2, 0, 1).reshape(C, 256)
        uvT_l = uv3[hs].transpose(2, 0, 1).reshape(C, 256)
        we = [qr3[h, 0::2, :] for h in hs]    # [32, C] each
        wo = [qr3[h, 1::2, :] for h in hs]
        wrA = np.concatenate([we[0], we[0], we[1], we[1]], 0).T  # [C, 128]
        wrB = np.concatenate([wo[0], wo[0], wo[1], wo[1]], 0).T
        be = [bqr2[h, 0::2] for h in hs]
        bo = [bqr2[h, 1::2] for h in hs]
        bA = np.concatenate([be[0], be[0], be[1], be[1]])[:, None]  # [128,1]
        bB = np.concatenate([bo[0], bo[0], bo[1], bo[1]])[:, None]
        in_maps.append({
            "xT_loc": pack_hid(xs).astype(BF),
            "wdkvT": wdkvT, "wdqT": wdqT, "bdkv": bdkv, "bdq": bdq,
            "ukT": pack_c(ukT_l).astype(BF),
            "uqT": pack_c(uqT_l).astype(BF),
            "uvT": pack_c(uvT_l).astype(BF),
            "buk": buk2[hs].reshape(2, 128, 1).copy(),
            "buq": buq2[hs].reshape(2, 128, 1).copy(),
            "buv": buv2[hs].reshape(1, 256).astype(BF),
            "wrAqT": pack_c(wrA * SCALE).astype(BF),
            "wrBqT": pack_c(wrB * SCALE).astype(BF),
            "wrAkT": pack_c(wrA).astype(BF),
            "wrBkT": pack_c(wrB).astype(BF),
            "bAq": (bA * SCALE).astype(np.float32),
            "bBq": (bB * SCALE).astype(np.float32),
            "bAk": bA.astype(np.float32),
            "bBk": bB.astype(np.float32),
            "c1k": (c1 * SK).astype(np.float16),
            "c2k": (c2 * SK).astype(np.float16),
            "c1q": (c1 * SQ).astype(np.float16),
            "c2q": (c2 * SQ).astype(np.float16),
            "woT": woT,
        })
    return in_maps


def kernel(**inputs):
    if "nc" not in _CACHE:
        _CACHE["nc"] = _build_program()
    nc = _CACHE["nc"]
    in_maps = _host_prep(inputs)
    res = run_bass_kernel_spmd(nc, in_maps, list(range(NCORES)))
    out = np.concatenate([res.results[j]["out_slice"] for j in range(NCORES)],
                         0)
    out = out + np.asarray(inputs["out_b"], np.float32)[None, :]
    return out.reshape(B, S, HID)
